# revision 1
# baseline (speedup 1.0000x reference)
"""Trainium2 Bass kernel for nn_MultiHeadAttention_69106023793143.

Reference computation (B=4, S=2048, D=1024, H=16, HD=64):
    qh = split_heads(q @ Wq + bq); kh, vh likewise
    out = merge_heads(sigmoid((qh @ kh^T) / sqrt(HD)) @ vh)

Sharding (8 cores): core c handles batch b = c//2 and the half = c%2 slice of
the feature axis (512 features = 8 heads).  Projections are tensor-parallel on
the output dim of Wq/Wk/Wv; attention is head-parallel.  The final [B,S,D]
output is assembled host-side from the per-core [512, 2048] transposed blocks.

Device strategy per core:
  - Host pre-transposes q/k/v to x^T [D, S] so the contraction dim (features)
    lands on SBUF partitions with plain contiguous DMAs — no on-device
    transposes anywhere.
  - Q^T, K^T computed as W^T-slice @ x^T -> [of, tok] layout (head dim on
    partitions), V computed natural [tok, of].
  - scores^T[k, q] = Kh^T.T @ Qh^T via row-tiled (K=64) matmul pairs: two
    heads run concurrently on disjoint PE row groups ((0,0)/(64,0)).
  - sigmoid on ScalarE directly from PSUM, one 2-bank wave (one k-tile x two
    heads, N=1024) per ACTIVATE, with the 1/sqrt(HD) scale folded into ACT's
    free affine. AV matmuls lag the sigmoid stream and accumulate out^T[d, q]
    over the 16 k-tiles into one [64,512] PSUM bank per head (dst partition
    base must be 0: col-tiled dst != 0 is rejected by this toolchain).
  - PSUM (8 banks): 2 score slots x 2 banks + 2 projection accumulators
    (vacc0/1) + 2 output accumulators (acc0/1). Dedicated projection
    accumulators let V / per-q-chunk Q projections pipeline into PE slack
    under the sigmoid stream. K and V inputs stream in 256-token
    half-chunks, and V half-chunks 2..7 are injected just-in-time into
    round 0's wave stream so V k-tiles keep just ahead of the lagging AV
    matmuls.
  - All matmuls run in float32r (fp32 storage, FP22 multiply) for full PE
    rate; PSUM accumulation is fp32. End-to-end max rel err ~2.5e-4.
  - Nonzero biases are folded in by augmenting the contraction dim with a
    ones-row (host-side, KT=9); with zero biases (the spec'd case) no
    padding is used.
"""

import sys

if "/opt/trn_rl_repo" not in sys.path:
    sys.path.insert(0, "/opt/trn_rl_repo")

from contextlib import ExitStack

import numpy as np

import concourse.tile as tile
from concourse import bacc, mybir
from concourse.bass_utils import run_bass_kernel_spmd

B, S, D, H = 4, 2048, 1024, 16
HD = D // H  # 64
OF = D // 2  # 512 features (8 heads) per core
N_CORES = 8
P = 128
TOK_T = S // P  # 16 token tiles
QC = S // 512  # 4 query chunks of 512
HP = 4  # head pairs per core
F32 = mybir.dt.float32
F32R = mybir.dt.float32r

# number of (kt, head) S-tile jobs per (head-pair, q-chunk), grouped in
# waves of 3 PSUM banks per ACTIVATE
WAVE = 2

# When True, the projection inputs (x^T and W) are shipped and multiplied in
# bfloat16: halves the serial prefix DMA (~27 MiB -> ~13.5 MiB) at the cost of
# ~10x higher (but still small) output error. Default off: fp32/float32r
# everywhere gives ~2.5e-4 max rel err.
BF16_INPUTS = False

_cache: dict = {}

# results of the most recent run (exec time etc.), for test harnesses
last_results = None


def _build(KT: int):
    """Build the SPMD Bass program. KT = contraction k-tiles (8, or 9 when
    biases are folded in via an augmented ones-row)."""
    nc = bacc.Bacc("TRN2", target_bir_lowering=False, debug=False,
                   num_devices=N_CORES, name="mha_sig")

    KA = KT * P  # augmented contraction size
    XDT = mybir.dt.bfloat16 if BF16_INPUTS else F32R
    xq = nc.dram_tensor("xq", [KA, S], XDT, kind="ExternalInput")
    xk = nc.dram_tensor("xk", [KA, S], XDT, kind="ExternalInput")
    xv = nc.dram_tensor("xv", [KA, S], XDT, kind="ExternalInput")
    wq = nc.dram_tensor("wq", [KA, OF], XDT, kind="ExternalInput")
    wk = nc.dram_tensor("wk", [KA, OF], XDT, kind="ExternalInput")
    wv = nc.dram_tensor("wv", [KA, OF], XDT, kind="ExternalInput")
    o_t = nc.dram_tensor("o_t", [OF, S], F32, kind="ExternalOutput")

    # the augmented (KT=9) layout is bigger; drop x-chunk buffering to fit
    xbufs = 3 if KT == 8 else 2

    xq_r = xq.rearrange("(kt p) t -> p kt t", p=P)
    xk_r = xk.rearrange("(kt p) t -> p kt t", p=P)
    xv_r = xv.rearrange("(kt p) t -> p kt t", p=P)

    with tile.TileContext(nc) as tc:
        with ExitStack() as ctx:
            persist = ctx.enter_context(tc.tile_pool(name="persist", bufs=1))
            wpool = ctx.enter_context(tc.tile_pool(name="wpool", bufs=1))
            xpool = ctx.enter_context(tc.tile_pool(name="xpool", bufs=2))
            ps_pool = ctx.enter_context(
                tc.tile_pool(name="ps_pool", bufs=2, space="PSUM"))
            apool = ctx.enter_context(tc.tile_pool(name="apool", bufs=3))
            opool = ctx.enter_context(tc.tile_pool(name="opool", bufs=1))

            # --- persistent weights + projection outputs ---
            # (each W is DMA'd right before the projection phase that uses it
            # so the serial prefix DMA stream isn't front-loaded with all
            # three weight tensors)
            wk_sb = persist.tile([P, KT, OF], XDT)
            nc.sync.dma_start(wk_sb[:], wk.rearrange("(kt p) n -> p kt n", p=P))
            wv_sb = persist.tile([P, KT, OF], XDT)
            wq_sb = persist.tile([P, KT, OF], XDT)

            # K^T / Q^T: [of-in-tile, of-tile, tok];  V: [tok-in-tile, kt, of]
            kt_sb = persist.tile([P, HP, S], F32R)
            v_sb = persist.tile([P, TOK_T, OF], F32R)

            def proj_transposed(x_r, w_sb, dst, tc_idx, label, width=512):
                """dst[:, m, tc*width:+width] = (W-slice).T @ x-chunk
                ([of, tok]); for label=="q", dst is a per-chunk [P, HP, 512]
                tile and the tok axis is not offset."""
                # q chunks get their own slot so the first q-chunk's DMA is
                # not serialized behind all the k chunks in the pool rotation
                x_tile = xpool.tile([P, KT, width], XDT,
                                    tag="xq" if label == "q" else "xchunk",
                                    bufs=1 if label == "q" else xbufs,
                                    name=f"x_{label}_{tc_idx}")
                nc.sync.dma_start(
                    x_tile[:],
                    x_r[:, :, tc_idx * width:(tc_idx + 1) * width])
                for m in range(HP):
                    ps = ps_pool.tile([P, width], F32, tag=f"vacc{m % 2}",
                                      bufs=1, name=f"ps_{label}_{tc_idx}_{m}")
                    for kt in range(KT):
                        nc.tensor.matmul(
                            ps[:],
                            lhsT=w_sb[:, kt, m * P:(m + 1) * P],
                            rhs=x_tile[:, kt, :],
                            start=(kt == 0),
                            stop=(kt == KT - 1),
                        )
                    if label == "q":
                        nc.vector.tensor_copy(out=dst[:, m, :], in_=ps[:])
                    else:
                        nc.vector.tensor_copy(
                            out=dst[:, m,
                                    tc_idx * width:(tc_idx + 1) * width],
                            in_=ps[:])

            def proj_v(hc_idx):
                """v_sb[:, hc*2+m, :] = x-half-chunk.T @ Wv  ([tok, of]);
                256-token half-chunks so V k-tiles land at finer granularity
                under round 0's sigmoid stream."""
                x_tile = xpool.tile([P, KT, 256], XDT, tag="xchunk", bufs=xbufs,
                                    name=f"x_v_{hc_idx}")
                nc.sync.dma_start(x_tile[:],
                                  xv_r[:, :, hc_idx * 256:(hc_idx + 1) * 256])
                for m in range(2):
                    ps = ps_pool.tile([P, 512], F32, tag=f"vacc{m % 2}", bufs=1,
                                      name=f"ps_v_{hc_idx}_{m}")
                    for kt in range(KT):
                        nc.tensor.matmul(
                            ps[:],
                            lhsT=x_tile[:, kt, m * P:(m + 1) * P],
                            rhs=wv_sb[:, kt, :],
                            start=(kt == 0),
                            stop=(kt == KT - 1),
                        )
                    nc.vector.tensor_copy(out=v_sb[:, hc_idx * 2 + m, :],
                                          in_=ps[:])

            # K projections first, then Q for the first q-chunk (these gate
            # the first sigmoid waves). V projections are emitted inside the
            # first attention q-chunk: they have their own PSUM tags (vacc*)
            # so they pipeline into PE slack while sigmoids run, and the
            # lagging AV matmuls pick up each V tile as it lands.
            for tc_idx in range(2 * QC):
                proj_transposed(xk_r, wk_sb, kt_sb, tc_idx, "k", width=256)
            nc.sync.dma_start(wq_sb[:], wq.rearrange("(kt p) n -> p kt n", p=P))
            qt_tiles = {}
            qt_tiles[0] = xpool.tile([P, HP, 512], F32R, tag="qt", bufs=2,
                                     name="qt_0")
            proj_transposed(xq_r, wq_sb, qt_tiles[0], 0, "q")
            nc.sync.dma_start(wv_sb[:], wv.rearrange("(kt p) n -> p kt n", p=P))

            # jobs per (hp, qc): (kt, head) pairs, kt-major so adjacent jobs
            # alternate PE row groups
            jobs = [(kt, h) for kt in range(TOK_T) for h in range(2)]
            waves = [jobs[i:i + WAVE] for i in range(0, len(jobs), WAVE)]

            # Only the first half of V precedes attention: the second half
            # streams in under round 0, staying just ahead of the lagging AV
            # matmuls (V tiles are produced ~1 per 1.8us vs consumed ~1 per
            # 1.1us, and the AVs start a few waves behind the sigmoids).
            proj_v(0)
            proj_v(1)

            for qc in range(QC):
                if qc > 0:
                    # just-in-time Q projection for the next q-chunk
                    qt_tiles[qc] = xpool.tile([P, HP, 512], F32R, tag="qt",
                                              bufs=2, name=f"qt_{qc}")
                    proj_transposed(xq_r, wq_sb, qt_tiles[qc], qc, "q")

                for hp in range(HP):
                    # per-head output accumulators, both at PSUM partition 0
                    # (col-tiled dst partitions != 0 are rejected by walrus
                    # ISA checks in this toolchain)
                    o_accs = [
                        ps_pool.tile([HD, 512], F32, tag=f"acc{h}", bufs=1,
                                     name=f"oacc{h}_{qc}_{hp}")
                        for h in range(2)
                    ]
                    def emit_avs(wave, a_t):
                        for j, (kt, h) in enumerate(wave):
                            # out^T[d, q] += V-tile.T @ attn^T-tile
                            nc.tensor.matmul(
                                o_accs[h][:],
                                lhsT=v_sb[:, kt,
                                          hp * P + h * HD:hp * P + (h + 1) * HD],
                                rhs=a_t[:, j, :],
                                start=(kt == 0),
                                stop=(kt == TOK_T - 1),
                            )

                    # AV matmuls are emitted one wave behind the scores so the
                    # in-order PE stream never blocks on the current wave's
                    # sigmoid (S(w+1) runs while ACT processes wave w).
                    pending = None
                    for wi, wave in enumerate(waves):
                        # V chunks tc1..tc3 are projected just-in-time inside
                        # round 0's wave stream (wave index == k-tile), so the
                        # lagging AV matmuls never outrank-starve the sigmoid
                        # pipeline
                        if qc == 0 and hp == 0 and wi in (0, 2, 4, 6, 8, 10):
                            proj_v(2 + wi // 2)
                        st = ps_pool.tile([P, WAVE, 512], F32, tag="scores",
                                       bufs=2, name=f"st_{qc}_{hp}_{wi}")
                        for j, (kt, h) in enumerate(wave):
                            # scores^T tile: [k-tokens, q-tokens] for head
                            # 2hp+h; contraction over d (64 rows)
                            nc.tensor.matmul(
                                st[:, j, :],
                                lhsT=kt_sb[h * HD:(h + 1) * HD, hp,
                                           kt * P:(kt + 1) * P],
                                rhs=qt_tiles[qc][h * HD:(h + 1) * HD, hp, :],
                                start=True,
                                stop=True,
                                tile_position=(h * HD, 0),
                            )
                        a_t = apool.tile([P, WAVE, 512], F32R, tag="a_t",
                                         name=f"a_{qc}_{hp}_{wi}")
                        nc.scalar.activation(
                            out=a_t[:, :len(wave), :],
                            in_=st[:, :len(wave), :],
                            func=mybir.ActivationFunctionType.Sigmoid,
                            scale=1.0 / np.sqrt(HD).item(),
                        )
                        if pending is not None:
                            emit_avs(*pending)
                        pending = (wave, a_t)
                    emit_avs(*pending)
                    o_sb = opool.tile([P, 512], F32, tag="o_sb",
                                      name=f"osb_{qc}_{hp}")
                    nc.vector.tensor_copy(out=o_sb[0:HD, :], in_=o_accs[0][:])
                    nc.vector.tensor_copy(out=o_sb[HD:P, :], in_=o_accs[1][:])
                    nc.sync.dma_start(
                        o_t[hp * P:(hp + 1) * P, qc * 512:(qc + 1) * 512],
                        o_sb[:])

    nc.compile()
    return nc


def _prep_core_inputs(q, k, v, Wq, bq, Wk, bk, Wv, bv, KT):
    """Host-side shard + transpose. Returns in_maps for 8 cores."""
    KA = KT * P
    aug = KA > D
    if BF16_INPUTS:
        import ml_dtypes
        xdt = ml_dtypes.bfloat16
    else:
        xdt = np.float32

    def x_t(x_b):  # [S, D] -> [KA, S]
        xt = np.ascontiguousarray(x_b.T)  # [D, S]
        if not aug:
            return xt.astype(xdt)
        out = np.zeros((KA, S), xdt)
        out[:D] = xt
        out[D] = 1.0
        return out

    def w_slice(W, b, half):  # -> [KA, OF]
        ws = W[:, half * OF:(half + 1) * OF]
        if not aug:
            return np.ascontiguousarray(ws).astype(xdt)
        out = np.zeros((KA, OF), xdt)
        out[:D] = ws
        out[D] = b[half * OF:(half + 1) * OF]
        return out

    xts = {}
    in_maps = []
    for c in range(N_CORES):
        b, half = divmod(c, 2)
        if b not in xts:
            xts[b] = (x_t(q[b]), x_t(k[b]), x_t(v[b]))
        xq_c, xk_c, xv_c = xts[b]
        in_maps.append({
            "xq": xq_c,
            "xk": xk_c,
            "xv": xv_c,
            "wq": w_slice(Wq, bq, half),
            "wk": w_slice(Wk, bk, half),
            "wv": w_slice(Wv, bv, half),
        })
    return in_maps


def kernel(q, k, v, Wq, bq, Wk, bk, Wv, bv):
    global last_results
    q = np.ascontiguousarray(np.asarray(q, np.float32))
    k = np.ascontiguousarray(np.asarray(k, np.float32))
    v = np.ascontiguousarray(np.asarray(v, np.float32))
    Wq = np.asarray(Wq, np.float32)
    Wk = np.asarray(Wk, np.float32)
    Wv = np.asarray(Wv, np.float32)
    bq = np.asarray(bq, np.float32)
    bk = np.asarray(bk, np.float32)
    bv = np.asarray(bv, np.float32)

    aug = any(np.any(b_) for b_ in (bq, bk, bv))
    KT = (D // P) + (1 if aug else 0)

    key = (KT, BF16_INPUTS)
    if key not in _cache:
        _cache[key] = _build(KT)
    nc = _cache[key]

    in_maps = _prep_core_inputs(q, k, v, Wq, bq, Wk, bk, Wv, bv, KT)
    res = run_bass_kernel_spmd(nc, in_maps, core_ids=list(range(N_CORES)))
    last_results = res

    out = np.empty((B, S, D), np.float32)
    for c in range(N_CORES):
        b, half = divmod(c, 2)
        out[b, :, half * OF:(half + 1) * OF] = res.results[c]["o_t"].T
    return out



# revision 19
# speedup vs baseline: 1.2785x; 1.2785x over previous
"""Trainium2 Bass kernel for nn_MultiHeadAttention_69106023793143.

Reference computation (B=4, S=2048, D=1024, H=16, HD=64):
    qh = split_heads(q @ Wq + bq); kh, vh likewise
    out = merge_heads(sigmoid((qh @ kh^T) / sqrt(HD)) @ vh)

Sharding (8 cores): core c handles batch b = c//2 and the half = c%2 slice of
the feature axis (512 features = 8 heads).  Projections are tensor-parallel on
the output dim of Wq/Wk/Wv; attention is head-parallel.

Device strategy per core (ACT-engine-paced pipeline):
  - The sigmoid over all 8*2048*2048 scores is the hard floor (ACT processes
    1 elem/cycle/partition @1.2GHz ~= 218us/core); everything else is
    scheduled to hide under the ACT stream.
  - Projections run as split-fp8 DoubleRow matmuls: host ships x and W as
    (hi, lo) fp8e4 pairs (same bytes as bf16) and the product takes the three
    cross terms xh*Wh + xh*Wl + xl*Wh - ~bf16 accuracy at 0.75x the bf16 PE
    cost.  W is host-scaled by 16 so its N(0, 1/1024) entries stay in e4m3's
    normal range; the 1/256 score scale folds into the sigmoid affine and a
    1/16 into the output copy.
  - Q^T/K^T land as [of, tok] with head PAIRS stacked on the 128 partitions,
    V natural [tok, of].
  - Scores use fp8e4 DoubleRow matmuls at 0.5 cycles/row: kh is stored as an
    (hi, lo) fp8 pair - the two DoubleRow K-blocks - so K-side quantization
    error is compensated; qh is plain fp8 broadcast across the two blocks
    (stride-0 AP).  Odd heads live on partitions 64:128 (tile_position
    (64, 0)).  End-to-end max rel err ~1.35e-2 (q-side fp8 only), vs the
    2e-2 budget.
  - Sigmoid on ScalarE in alternating 3-bank/2-bank PSUM waves, writing attn
    directly as bf16 to SBUF.
  - Attention runs in rounds of (head, q-chunk-PAIR) with the k-token axis
    OUTERMOST inside a round: jobs (h, qcp, kt, i).  A round therefore takes
    ~13 ACT waves to sweep the k tokens, which spreads the xk-chunk DMA and
    V-projection deadlines far enough apart that the serial DMA engines and
    the PE projection stream keep up with ACT from the start (a (h, qc)
    round with kt innermost sweeps all 2048 k tokens in 6 waves - the DMA
    can't feed that).
  - AV in bf16 with out[q, d] layout: lhsT = attn^T tile [128k, 128q],
    rhs = v [128k, 64d] -> only 64 free rows per matmul.  Each round
    accumulates its 8 q-tiles (2 q-chunks x 4) as interleaved sub-bank
    groups in ONE PSUM bank: start=True only on the round's first matmul
    (the PSUM zero-region covers the whole 2KB bank and zeroes on first
    touch per byte), stop=True on the last.
  - PSUM: 3+2 score banks + 2 projection banks (double-buffered so the
    projection pipeline never serializes on its DVE drain) + 1 AV bank = 8.
  - Wk/Wq are shipped p-major ([NHP, P, KTC*2*128]) so a head-pair slice is
    one contiguous 2KB-per-partition DMA; input DMAs are emitted up front in
    deadline order (the DMA engines are a serial resource in practice).
  - AV matmuls drain from a FIFO gated on their V-tile's emission so the
    in-order PE queue never head-of-line blocks on a V projection.
  - Nonzero biases fold in via a host-side augmented ones-row (KTC=9).
"""

import sys

if "/opt/trn_rl_repo" not in sys.path:
    sys.path.insert(0, "/opt/trn_rl_repo")

from collections import deque
from contextlib import ExitStack

import numpy as np

import concourse.tile as tile
from concourse import bacc, mybir
from concourse.bass_utils import run_bass_kernel_spmd

B, S, D, H = 4, 2048, 1024, 16
HD = D // H  # 64
OF = D // 2  # 512 features (8 heads) per core
N_CORES = 8
P = 128
NH = 8          # heads per core
NHP = 4         # head pairs per core
QC = 4          # q-chunks of 512
NQCP = 2        # q-chunk pairs
NKT = 16        # k token tiles of 128
TOKC = 4        # x token chunks of 512
RJOBS = 2 * NKT  # jobs per round (2 q-chunks x 16 kt)
ABUFS = 18      # attn (a_t) wave buffers
WS = 16.0       # host-side W scale (keeps fp8 W out of subnormals)

F32 = mybir.dt.float32
BF16 = mybir.dt.bfloat16
FP8 = mybir.dt.float8e4

# the three split-fp8 cross terms (w level, x level)
TERMS = ((0, 0), (0, 1), (1, 0))

_cache: dict = {}
last_results = None


def _build(KTC: int):
    """KTC = contraction k-tiles for the projections (8, or 9 when biases are
    folded in via an augmented ones-row)."""
    nc = bacc.Bacc("TRN2", target_bir_lowering=False, debug=False,
                   num_devices=N_CORES, name="mha_sig4")
    KA = KTC * P
    NDR = KTC // 2   # DoubleRow kt-pairs per term
    AUGK = KTC % 2   # leftover kt (the ones-row) as plain fp8 matmul
    WFREE = KTC * 2 * P  # per-partition elements of one head-pair W slice

    xq = nc.dram_tensor("xq", [KA, 2, S], FP8, kind="ExternalInput")
    xk = nc.dram_tensor("xk", [KA, 2, S], FP8, kind="ExternalInput")
    xv = nc.dram_tensor("xv", [KA, 2, S], FP8, kind="ExternalInput")
    # wq/wk p-major: [head-pair, partition, kt*level*128]
    wq = nc.dram_tensor("wq", [NHP, P, WFREE], FP8, kind="ExternalInput")
    wk = nc.dram_tensor("wk", [NHP, P, WFREE], FP8, kind="ExternalInput")
    wv = nc.dram_tensor("wv", [KA, 2, OF], FP8, kind="ExternalInput")
    o = nc.dram_tensor("o", [S, OF], F32, kind="ExternalOutput")

    xq_r = xq.rearrange("(kt p) l t -> p kt l t", p=P)
    xk_r = xk.rearrange("(kt p) l t -> p kt l t", p=P)
    xv_r = xv.rearrange("(kt p) l t -> p kt l t", p=P)
    wv_r = wv.rearrange("(kt p) l n -> p kt l n", p=P)

    abufs = ABUFS if KTC == 8 else 10

    with tile.TileContext(nc) as tc:
        with ExitStack() as ctx:
            persist = ctx.enter_context(tc.tile_pool(name="persist", bufs=1))
            xvpool = ctx.enter_context(tc.tile_pool(name="xvpool", bufs=2))
            apool = ctx.enter_context(tc.tile_pool(name="apool", bufs=abufs))
            opool = ctx.enter_context(tc.tile_pool(name="opool", bufs=2))
            ps_pool = ctx.enter_context(
                tc.tile_pool(name="ps_pool", bufs=2, space="PSUM"))

            wk_sb = persist.tile([P, NHP, KTC, 2, P], FP8)
            wq_sb = persist.tile([P, NHP, KTC, 2, P], FP8)
            wv_sb = persist.tile([P, KTC, 2, OF], FP8)
            xk_sb = persist.tile([P, KTC, 2, S], FP8)
            xq_sb = persist.tile([P, KTC, 2, S], FP8)
            # kh as (hi, lo) fp8 pair, head pairs stacked on partitions;
            # qh plain fp8; v bf16 [tok, of]
            kh = persist.tile([P, NHP, 2, S], FP8)
            qh = persist.tile([P, NHP, S], FP8)
            v_sb = persist.tile([P, NKT, OF], BF16)

            # ---------- producer closures ----------
            def dma_w_hp(w_sb, w_dram, hp):
                def run():
                    nc.sync.dma_start(
                        w_sb[:, hp].rearrange("p kt l n -> p (kt l n)"),
                        w_dram[hp])
                return run

            def dma_wv():
                def run():
                    nc.sync.dma_start(wv_sb[:], wv_r)
                return run

            def dma_x(x_sb, x_r, lo, hi):
                def run():
                    for lv in range(2):
                        nc.sync.dma_start(
                            x_sb[:, :, lv, lo:hi],
                            x_r[:, :, lv, lo:hi])
                return run

            xv_tiles = {}

            def dma_xv(c):
                def run():
                    t = xvpool.tile([P, KTC, 2, 512], FP8, tag="xvchunk",
                                    name=f"xv_{c}")
                    for lv in range(2):
                        nc.sync.dma_start(
                            t[:, :, lv, :],
                            xv_r[:, :, lv, c * 512:(c + 1) * 512])
                    xv_tiles[c] = t
                return run

            # warm-up: the PE runs at 0.65/1.2GHz until ~3us of continuous
            # execution; burn dummy matmuls during the prefix DMAs so the
            # first projections run at full clock
            wu_sb = persist.tile([HD, 2, 640], FP8)

            def warmup(n_mm):
                def run():
                    if n_mm < 0:
                        nc.vector.memset(wu_sb[:], 0)
                        return
                    st = ps_pool.tile([P, 3, 512], F32, tag="st3", bufs=1,
                                      name=f"wu_{n_mm}")
                    for m in range(n_mm):
                        nc.tensor.matmul(
                            st[:, m % 3, :],
                            lhsT=wu_sb[:, :, 0:P],
                            rhs=wu_sb[:, :, P:P + 512],
                            start=True, stop=True,
                            perf_mode=mybir.MatmulPerfMode.DoubleRow,
                            skip_group_check=True,
                        )
                return run

            # split-emission prefix projections: the hi terms only need the
            # lv0 (hi) half of the x chunk, so they start ~1.5us earlier
            _prefix_ps = {}

            def proj_kq_hi(x_sb, w_sb, hp, c, key):
                def run():
                    ps = ps_pool.tile([P, 512], F32, tag="proj", bufs=2,
                                      name=f"pjh_{key}")
                    _prefix_ps[key] = ps
                    n = 0
                    for lw, lx in ((0, 0), (1, 0)):
                        for t in range(NDR):
                            nc.tensor.matmul(
                                ps[:],
                                lhsT=w_sb[:, hp, 2 * t:2 * t + 2, lw, :],
                                rhs=x_sb[:, 2 * t:2 * t + 2, lx,
                                         c * 512:(c + 1) * 512],
                                start=(n == 0), stop=False,
                                perf_mode=mybir.MatmulPerfMode.DoubleRow,
                                skip_group_check=True,
                            )
                            n += 1
                return run

            def proj_kq_lo(x_sb, w_sb, hp, c, dst, split_lo, key):
                def run():
                    ps = _prefix_ps.pop(key)
                    for t in range(NDR):
                        nc.tensor.matmul(
                            ps[:],
                            lhsT=w_sb[:, hp, 2 * t:2 * t + 2, 0, :],
                            rhs=x_sb[:, 2 * t:2 * t + 2, 1,
                                     c * 512:(c + 1) * 512],
                            start=False, stop=(t == NDR - 1),
                            perf_mode=mybir.MatmulPerfMode.DoubleRow,
                            skip_group_check=True,
                        )
                    sl = slice(c * 512, (c + 1) * 512)
                    if split_lo:
                        nc.vector.tensor_copy(out=dst[:, hp, 0, sl], in_=ps[:])
                        nc.vector.tensor_sub(dst[:, hp, 1, sl], ps[:],
                                             dst[:, hp, 0, sl])
                    else:
                        nc.vector.tensor_copy(out=dst[:, hp, sl], in_=ps[:])
                return run

            def dma_x_lv(x_sb, x_r, lo, hi, lv):
                def run():
                    nc.sync.dma_start(
                        x_sb[:, :, lv, lo:hi],
                        x_r[:, :, lv, lo:hi])
                return run

            def proj_kq(x_sb, w_sb, hp, c, dst, split_lo):
                """dst slice [of-pair, tok chunk c] for head-pair hp."""
                def run():
                    ps = ps_pool.tile([P, 512], F32, tag="proj", bufs=2,
                                      name=f"pj_{'k' if split_lo else 'q'}"
                                           f"_{hp}_{c}")
                    n, last = 0, 3 * (NDR + AUGK) - 1
                    for lw, lx in TERMS:
                        for t in range(NDR):
                            nc.tensor.matmul(
                                ps[:],
                                lhsT=w_sb[:, hp, 2 * t:2 * t + 2, lw, :],
                                rhs=x_sb[:, 2 * t:2 * t + 2, lx,
                                         c * 512:(c + 1) * 512],
                                start=(n == 0), stop=(n == last),
                                perf_mode=mybir.MatmulPerfMode.DoubleRow,
                                skip_group_check=True,
                            )
                            n += 1
                        if AUGK:
                            nc.tensor.matmul(
                                ps[:],
                                lhsT=w_sb[:, hp, KTC - 1, lw, :],
                                rhs=x_sb[:, KTC - 1, lx,
                                         c * 512:(c + 1) * 512],
                                start=(n == 0), stop=(n == last),
                                skip_group_check=True,
                            )
                            n += 1
                    sl = slice(c * 512, (c + 1) * 512)
                    if split_lo:
                        nc.vector.tensor_copy(out=dst[:, hp, 0, sl], in_=ps[:])
                        nc.vector.tensor_sub(dst[:, hp, 1, sl], ps[:],
                                             dst[:, hp, 0, sl])
                    else:
                        nc.vector.tensor_copy(out=dst[:, hp, sl], in_=ps[:])
                return run

            def proj_v(t):
                """v_sb[:, t, :] = x-token-tile t @ Wv ([tok, of])."""
                def run():
                    ps = ps_pool.tile([P, 512], F32, tag="proj", bufs=2,
                                      name=f"pj_v_{t}")
                    xt = xv_tiles[t // 4]
                    tsl = slice((t % 4) * P, (t % 4 + 1) * P)
                    n, last = 0, 3 * (NDR + AUGK) - 1
                    for lw, lx in TERMS:
                        for u in range(NDR):
                            nc.tensor.matmul(
                                ps[:],
                                lhsT=xt[:, 2 * u:2 * u + 2, lx, tsl],
                                rhs=wv_sb[:, 2 * u:2 * u + 2, lw, :],
                                start=(n == 0), stop=(n == last),
                                perf_mode=mybir.MatmulPerfMode.DoubleRow,
                                skip_group_check=True,
                            )
                            n += 1
                        if AUGK:
                            nc.tensor.matmul(
                                ps[:],
                                lhsT=xt[:, KTC - 1, lx, tsl],
                                rhs=wv_sb[:, KTC - 1, lw, :],
                                start=(n == 0), stop=(n == last),
                                skip_group_check=True,
                            )
                            n += 1
                    nc.vector.tensor_copy(out=v_sb[:, t, :], in_=ps[:])
                return run

            # ---------- static schedule ----------
            # jobs: rounds of (h, qcp), k-token axis outermost inside the
            # round: (h, qc = 2*qcp + i, kt); job = h*64 + qcp*32 + kt*2 + i.
            # Round 0 staggers its second q-chunk by 2 k-tiles so the first
            # waves only need qh[qc0] (whose projection finishes first).
            jobs = []
            for kt in range(NKT + 2):
                if kt < NKT:
                    jobs.append((0, 0, kt))
                if kt >= 2:
                    jobs.append((0, 1, kt - 2))
            jobs += [(h, 2 * qcp + i, kt)
                     for h in range(NH) for qcp in range(NQCP)
                     for kt in range(NKT) for i in range(2)
                     if not (h == 0 and qcp == 0)]
            waves = []
            i0 = 0
            size = 3
            while i0 < len(jobs):
                waves.append(jobs[i0:i0 + size])
                i0 += size
                size = 5 - size  # alternate 3, 2

            producers = []  # (due_job, closure, vtile_or_None)
            # prefix + all input DMAs in deadline order (the DMA engines are
            # effectively serial; emission order = transfer order)
            if AUGK == 0:
                producers += [
                    (-99.9, warmup(-1), None),
                    (-99.8, warmup(10), None),
                    (-99.0, dma_w_hp(wk_sb, wk, 0), None),
                    (-98.9, dma_x_lv(xk_sb, xk_r, 0, 512, 0), None),
                    (-98.8, dma_w_hp(wq_sb, wq, 0), None),
                    (-98.7, proj_kq_hi(xk_sb, wk_sb, 0, 0, "k00"), None),
                    (-98.6, dma_x_lv(xk_sb, xk_r, 0, 512, 1), None),
                    (-98.5, proj_kq_lo(xk_sb, wk_sb, 0, 0, kh, True, "k00"),
                     None),
                    (-98.4, dma_x_lv(xq_sb, xq_r, 0, 512, 0), None),
                    (-98.3, warmup(4), None),
                    (-98.2, proj_kq_hi(xq_sb, wq_sb, 0, 0, "q00"), None),
                    (-98.1, dma_x_lv(xq_sb, xq_r, 0, 512, 1), None),
                    (-98.0, proj_kq_lo(xq_sb, wq_sb, 0, 0, qh, False, "q00"),
                     None),
                    (-97.9, dma_x_lv(xq_sb, xq_r, 512, 1024, 0), None),
                    (-97.8, proj_kq_hi(xq_sb, wq_sb, 0, 1, "q01"), None),
                    (-97.7, dma_x_lv(xq_sb, xq_r, 512, 1024, 1), None),
                    (-97.6, proj_kq_lo(xq_sb, wq_sb, 0, 1, qh, False, "q01"),
                     None),
                ]
            else:
                producers += [
                    (-99.0, dma_w_hp(wk_sb, wk, 0), None),
                    (-98.8, dma_x(xk_sb, xk_r, 0, 512), None),
                    (-98.6, dma_w_hp(wq_sb, wq, 0), None),
                    (-98.4, dma_x(xq_sb, xq_r, 0, 512), None),
                    (-98.2, proj_kq(xk_sb, wk_sb, 0, 0, kh, True), None),
                    (-98.0, proj_kq(xq_sb, wq_sb, 0, 0, qh, False), None),
                    (-97.8, dma_x(xq_sb, xq_r, 512, 1024), None),
                    (-97.6, proj_kq(xq_sb, wq_sb, 0, 1, qh, False), None),
                ]
            # earliest-deadline-first input stream; K(0, c1/c2) also use
            # split emission so kh is ready ~1.5us after the lv0 half lands
            if AUGK == 0:
                producers += [
                    (-89.8, dma_x_lv(xk_sb, xk_r, 512, 1024, 0), None),
                    (-89.7, proj_kq_hi(xk_sb, wk_sb, 0, 1, "k01"), None),
                    (-89.6, dma_x_lv(xk_sb, xk_r, 512, 1024, 1), None),
                    (-89.5, proj_kq_lo(xk_sb, wk_sb, 0, 1, kh, True, "k01"),
                     None),
                    (-88.8, dma_x_lv(xk_sb, xk_r, 1024, 1536, 0), None),
                    (-88.7, proj_kq_hi(xk_sb, wk_sb, 0, 2, "k02"), None),
                    (-88.6, dma_x_lv(xk_sb, xk_r, 1024, 1536, 1), None),
                    (-88.5, proj_kq_lo(xk_sb, wk_sb, 0, 2, kh, True, "k02"),
                     None),
                ]
            else:
                producers.append((-89, dma_x(xk_sb, xk_r, 512, 1024), None))
                producers.append((-88, dma_x(xk_sb, xk_r, 1024, 1536), None))
                producers.append((2, proj_kq(xk_sb, wk_sb, 0, 1, kh, True),
                                  None))
                producers.append((10, proj_kq(xk_sb, wk_sb, 0, 2, kh, True),
                                  None))
            producers.append((-87, dma_x(xk_sb, xk_r, 1536, 2048), None))
            producers.append((-86, dma_x(xq_sb, xq_r, 1024, 1536), None))
            producers.append((-85, dma_x(xq_sb, xq_r, 1536, 2048), None))
            producers.append((-84, dma_wv(), None))
            producers.append((-83, dma_xv(0), None))
            producers.append((-82, dma_xv(1), None))
            producers.append((10, dma_xv(2), None))
            producers.append((20, dma_xv(3), None))
            # kh chunk c3 needed from kt 12 (job ~25)
            producers.append((18, proj_kq(xk_sb, wk_sb, 0, 3, kh, True),
                              None))
            # Q projections for qc2/3: needed from job 32
            producers.append((24, proj_kq(xq_sb, wq_sb, 0, 2, qh, False),
                              None))
            producers.append((25, proj_kq(xq_sb, wq_sb, 0, 3, qh, False),
                              None))
            # V tiles: paced behind their xv chunk's DMA slot
            for t in range(NKT):
                producers.append((42 + (t // 4) * 6 + (t % 4),
                                  proj_v(t), t))
            for hp in range(1, NHP):
                base = 128 * hp
                producers.append((base - 64, dma_w_hp(wk_sb, wk, hp), None))
                producers.append((base - 62, dma_w_hp(wq_sb, wq, hp), None))
                for c in range(TOKC):
                    producers.append((base + 8 * c - 8,
                                      proj_kq(xk_sb, wk_sb, hp, c, kh, True),
                                      None))
                for qc in range(QC):
                    producers.append((base + 32 * (qc // 2) - 8 + (qc % 2),
                                      proj_kq(xq_sb, wq_sb, hp, qc, qh,
                                              False), None))
            producers.sort(key=lambda e: e[0])
            producers = deque(producers)
            v_emit_wave = {}

            # AV bookkeeping
            av_fifo = deque()  # (job_idx, h, qc, kt, a_t, j_in_wave, wave)
            av_state = {"tile": None, "round": -1}

            def finalize_round(r):
                av = av_state["tile"]
                h, qcp = divmod(r, NQCP)
                o_sb = opool.tile([P, 2, QC, HD], F32, tag="o_sb",
                                  name=f"osb_{r}")
                nc.vector.tensor_scalar_mul(
                    o_sb[:],
                    av[:].rearrange("p (i qt d) -> p i qt d", i=2, qt=QC),
                    1.0 / WS)
                for i in range(2):
                    qc = 2 * qcp + i
                    dst = o[qc * 512:(qc + 1) * 512,
                            h * HD:(h + 1) * HD].rearrange(
                                "(qt p) d -> p qt d", p=P)
                    nc.sync.dma_start(dst, o_sb[:, i])
                av_state["tile"] = None

            def drain_avs(cur_wave, final=False):
                budget = 6  # cap per-wave AV emission so a backlog burst
                # never parks in front of the score stream in the in-order
                # PE queue
                while av_fifo:
                    job, h, qc, kt, a_t, j, w = av_fifo[0]
                    if not final:
                        if budget <= 0:
                            break
                        if w >= cur_wave:
                            break
                        vw = v_emit_wave.get(kt)
                        if vw is None or vw >= cur_wave:
                            break
                        budget -= 1
                    av_fifo.popleft()
                    r = job // RJOBS
                    if r != av_state["round"]:
                        if av_state["tile"] is not None:
                            finalize_round(av_state["round"])
                        av_state["tile"] = ps_pool.tile(
                            [P, 512], F32, tag="av", bufs=1, name=f"av_{r}")
                        av_state["round"] = r
                    av = av_state["tile"]
                    i = qc % 2
                    first = (kt == 0 and i == 0)
                    last = (kt == NKT - 1 and i == 1)
                    for qt in range(4):
                        nc.tensor.matmul(
                            av[:, (i * 4 + qt) * HD:(i * 4 + qt + 1) * HD],
                            lhsT=a_t[:, j, qt * P:(qt + 1) * P],
                            rhs=v_sb[:, kt, h * HD:(h + 1) * HD],
                            start=(first and qt == 0),
                            stop=(last and qt == 3),
                            skip_group_check=True,
                        )

            # ---------- main wave loop ----------
            def drain_producers(w, job_base):
                while producers and producers[0][0] <= job_base + 2:
                    due, closure, vtile = producers.popleft()
                    closure()
                    if vtile is not None:
                        v_emit_wave[vtile] = w

            job_base = 0
            for w, wave in enumerate(waves):
                if w == 0:
                    drain_producers(w, job_base)
                g = len(wave)
                st = ps_pool.tile([P, g, 512], F32, tag=f"st{g}", bufs=1,
                                  name=f"st_{w}")
                for j, (h, qc, kt) in enumerate(wave):
                    hp, pb = h // 2, (h % 2) * HD
                    lhsT = kh[pb:pb + HD, hp, :, kt * P:(kt + 1) * P]
                    for half in range(2):
                        rhs = qh[pb:pb + HD, hp,
                                 qc * 512 + half * 256:
                                 qc * 512 + (half + 1) * 256]
                        rhs = rhs.unsqueeze(1).broadcast_to([HD, 2, 256])
                        nc.tensor.matmul(
                            st[:, j, half * 256:(half + 1) * 256],
                            lhsT=lhsT,
                            rhs=rhs,
                            start=True,
                            stop=True,
                            perf_mode=mybir.MatmulPerfMode.DoubleRow,
                            tile_position=(pb, 0),
                            skip_group_check=True,
                        )
                a_t = apool.tile([P, 3, 512], BF16, tag="a_t", name=f"a_{w}")
                nc.scalar.activation(
                    out=a_t[:, :g, :],
                    in_=st[:],
                    func=mybir.ActivationFunctionType.Sigmoid,
                    scale=0.125 / (WS * WS),
                )
                for j, (h, qc, kt) in enumerate(wave):
                    av_fifo.append((h * 64 + (qc // 2) * 32 + kt * 2
                                    + (qc % 2), h, qc, kt, a_t, j, w))
                drain_avs(w)
                job_base += g
            while producers:
                producers.popleft()[1]()
            drain_avs(0, final=True)
            finalize_round(av_state["round"])

    nc.compile()
    return nc


def _prep_core_inputs(q, k, v, Wq, bq, Wk, bk, Wv, bv, KTC):
    """Host-side shard + transpose + split-fp8 packing. in_maps for 8 cores."""
    import ml_dtypes
    E4 = ml_dtypes.float8_e4m3
    KA = KTC * P
    aug = KA > D

    def split8(a):
        """[R, C] fp32 -> [R, 2, C] fp8 (hi, lo)."""
        hi = a.astype(E4)
        lo = (a - hi.astype(np.float32)).astype(E4)
        return np.ascontiguousarray(np.stack([hi, lo], axis=1))

    def x_t(x_b):  # [S, D] -> [KA, 2, S] fp8
        xt = np.ascontiguousarray(x_b.T)
        if aug:
            pad = np.zeros((KA, S), np.float32)
            pad[:D] = xt
            pad[D] = 1.0
            xt = pad
        return split8(xt)

    def w_kq(W, b, half):  # -> [NHP, P, KTC*2*128] fp8, p-major
        ws = np.ascontiguousarray(W[:, half * OF:(half + 1) * OF]) * WS
        if aug:
            pad = np.zeros((KA, OF), np.float32)
            pad[:D] = ws
            pad[D] = b[half * OF:(half + 1) * OF] * WS
            ws = pad
        s8 = split8(ws)  # [KA, 2, OF]
        pm = s8.reshape(KTC, P, 2, NHP, P).transpose(3, 1, 0, 2, 4)
        return np.ascontiguousarray(pm.reshape(NHP, P, KTC * 2 * P))

    def w_v(W, b, half):  # -> [KA, 2, OF] fp8
        ws = np.ascontiguousarray(W[:, half * OF:(half + 1) * OF]) * WS
        if aug:
            pad = np.zeros((KA, OF), np.float32)
            pad[:D] = ws
            pad[D] = b[half * OF:(half + 1) * OF] * WS
            ws = pad
        return split8(ws)

    xts = {}
    in_maps = []
    for c in range(N_CORES):
        b, half = divmod(c, 2)
        if b not in xts:
            xts[b] = (x_t(q[b]), x_t(k[b]), x_t(v[b]))
        xq_c, xk_c, xv_c = xts[b]
        in_maps.append({
            "xq": xq_c,
            "xk": xk_c,
            "xv": xv_c,
            "wq": w_kq(Wq, bq, half),
            "wk": w_kq(Wk, bk, half),
            "wv": w_v(Wv, bv, half),
        })
    return in_maps


def kernel(q, k, v, Wq, bq, Wk, bk, Wv, bv):
    global last_results
    q = np.ascontiguousarray(np.asarray(q, np.float32))
    k = np.ascontiguousarray(np.asarray(k, np.float32))
    v = np.ascontiguousarray(np.asarray(v, np.float32))
    Wq = np.asarray(Wq, np.float32)
    Wk = np.asarray(Wk, np.float32)
    Wv = np.asarray(Wv, np.float32)
    bq = np.asarray(bq, np.float32)
    bk = np.asarray(bk, np.float32)
    bv = np.asarray(bv, np.float32)

    aug = any(np.any(b_) for b_ in (bq, bk, bv))
    KTC = (D // P) + (1 if aug else 0)

    if KTC not in _cache:
        _cache[KTC] = _build(KTC)
    nc = _cache[KTC]

    in_maps = _prep_core_inputs(q, k, v, Wq, bq, Wk, bk, Wv, bv, KTC)
    res = run_bass_kernel_spmd(nc, in_maps, core_ids=list(range(N_CORES)))
    last_results = res

    out = np.empty((B, S, D), np.float32)
    for c in range(N_CORES):
        b, half = divmod(c, 2)
        out[b, :, half * OF:(half + 1) * OF] = res.results[c]["o"]
    return out


# revision 26
# speedup vs baseline: 1.2801x; 1.0013x over previous
"""Trainium2 Bass kernel for nn_MultiHeadAttention_69106023793143.

Reference computation (B=4, S=2048, D=1024, H=16, HD=64):
    qh = split_heads(q @ Wq + bq); kh, vh likewise
    out = merge_heads(sigmoid((qh @ kh^T) / sqrt(HD)) @ vh)

Sharding (8 cores): core c handles batch b = c//2 and the half = c%2 slice of
the feature axis (512 features = 8 heads).  Projections are tensor-parallel on
the output dim of Wq/Wk/Wv; attention is head-parallel.

Device strategy per core (ACT-engine-paced pipeline):
  - The sigmoid over all 8*2048*2048 scores is the hard floor (ACT processes
    1 elem/cycle/partition @1.2GHz ~= 218us/core); everything else is
    scheduled to hide under the ACT stream.
  - Projections run as split-fp8 DoubleRow matmuls: host ships x and W as
    (hi, lo) fp8e4 pairs (same bytes as bf16) and the product takes the three
    cross terms xh*Wh + xh*Wl + xl*Wh - ~bf16 accuracy at 0.75x the bf16 PE
    cost.  W is host-scaled by 16 so its N(0, 1/1024) entries stay in e4m3's
    normal range; the 1/256 score scale folds into the sigmoid affine and a
    1/16 into the output copy.
  - Q^T/K^T land as [of, tok] with head PAIRS stacked on the 128 partitions,
    V natural [tok, of].
  - Scores use fp8e4 DoubleRow matmuls at 0.5 cycles/row: kh is stored as an
    (hi, lo) fp8 pair - the two DoubleRow K-blocks - so K-side quantization
    error is compensated; qh is plain fp8 broadcast across the two blocks
    (stride-0 AP).  Odd heads live on partitions 64:128 (tile_position
    (64, 0)).  End-to-end max rel err ~1.35e-2 (q-side fp8 only), vs the
    2e-2 budget.
  - Sigmoid on ScalarE in alternating 3-bank/2-bank PSUM waves, writing attn
    directly as bf16 to SBUF.
  - Attention runs in rounds of (head, q-chunk-PAIR) with the k-token axis
    OUTERMOST inside a round: jobs (h, qcp, kt, i).  A round therefore takes
    ~13 ACT waves to sweep the k tokens, which spreads the xk-chunk DMA and
    V-projection deadlines far enough apart that the serial DMA engines and
    the PE projection stream keep up with ACT from the start (a (h, qc)
    round with kt innermost sweeps all 2048 k tokens in 6 waves - the DMA
    can't feed that).
  - AV in bf16 with out[q, d] layout: lhsT = attn^T tile [128k, 128q],
    rhs = v [128k, 64d] -> only 64 free rows per matmul.  Each round
    accumulates its 8 q-tiles (2 q-chunks x 4) as interleaved sub-bank
    groups in ONE PSUM bank: start=True only on the round's first matmul
    (the PSUM zero-region covers the whole 2KB bank and zeroes on first
    touch per byte), stop=True on the last.
  - PSUM: 3+2 score banks + 2 projection banks (double-buffered so the
    projection pipeline never serializes on its DVE drain) + 1 AV bank = 8.
  - Wk/Wq are shipped p-major ([NHP, P, KTC*2*128]) so a head-pair slice is
    one contiguous 2KB-per-partition DMA; input DMAs are emitted up front in
    deadline order (the DMA engines are a serial resource in practice).
  - AV matmuls drain from a FIFO gated on their V-tile's emission so the
    in-order PE queue never head-of-line blocks on a V projection.
  - Nonzero biases fold in via a host-side augmented ones-row (KTC=9).
"""

import sys

if "/opt/trn_rl_repo" not in sys.path:
    sys.path.insert(0, "/opt/trn_rl_repo")

from collections import deque
from contextlib import ExitStack

import numpy as np

import concourse.tile as tile
from concourse import bacc, mybir
from concourse.bass_utils import run_bass_kernel_spmd

B, S, D, H = 4, 2048, 1024, 16
HD = D // H  # 64
OF = D // 2  # 512 features (8 heads) per core
N_CORES = 8
P = 128
NH = 8          # heads per core
NHP = 4         # head pairs per core
QC = 4          # q-chunks of 512
NQCP = 2        # q-chunk pairs
NKT = 16        # k token tiles of 128
TOKC = 4        # x token chunks of 512
RJOBS = 2 * NKT  # jobs per round (2 q-chunks x 16 kt)
ABUFS = 18      # attn (a_t) wave buffers
WS = 16.0       # host-side W scale (keeps fp8 W out of subnormals)

F32 = mybir.dt.float32
BF16 = mybir.dt.bfloat16
FP8 = mybir.dt.float8e4

# the three split-fp8 cross terms (w level, x level)
TERMS = ((0, 0), (0, 1), (1, 0))

_cache: dict = {}
last_results = None


def _build(KTC: int):
    """KTC = contraction k-tiles for the projections (8, or 9 when biases are
    folded in via an augmented ones-row)."""
    nc = bacc.Bacc("TRN2", target_bir_lowering=False, debug=False,
                   num_devices=N_CORES, name="mha_sig4")
    KA = KTC * P
    NDR = KTC // 2   # DoubleRow kt-pairs per term
    AUGK = KTC % 2   # leftover kt (the ones-row) as plain fp8 matmul
    WFREE = KTC * 2 * P  # per-partition elements of one head-pair W slice

    xq = nc.dram_tensor("xq", [KA, 2, S], FP8, kind="ExternalInput")
    xk = nc.dram_tensor("xk", [KA, 2, S], FP8, kind="ExternalInput")
    xv = nc.dram_tensor("xv", [KA, 2, S], FP8, kind="ExternalInput")
    # wq/wk p-major: [head-pair, partition, kt*level*128]
    wq = nc.dram_tensor("wq", [NHP, P, WFREE], FP8, kind="ExternalInput")
    wk = nc.dram_tensor("wk", [NHP, P, WFREE], FP8, kind="ExternalInput")
    wv = nc.dram_tensor("wv", [KA, 2, OF], FP8, kind="ExternalInput")
    o = nc.dram_tensor("o", [S, OF], F32, kind="ExternalOutput")

    xq_r = xq.rearrange("(kt p) l t -> p kt l t", p=P)
    xk_r = xk.rearrange("(kt p) l t -> p kt l t", p=P)
    xv_r = xv.rearrange("(kt p) l t -> p kt l t", p=P)
    wv_r = wv.rearrange("(kt p) l n -> p kt l n", p=P)

    abufs = ABUFS if KTC == 8 else 10

    with tile.TileContext(nc) as tc:
        with ExitStack() as ctx:
            persist = ctx.enter_context(tc.tile_pool(name="persist", bufs=1))
            xvpool = ctx.enter_context(tc.tile_pool(name="xvpool", bufs=2))
            apool = ctx.enter_context(tc.tile_pool(name="apool", bufs=abufs))
            opool = ctx.enter_context(tc.tile_pool(name="opool", bufs=2))
            ps_pool = ctx.enter_context(
                tc.tile_pool(name="ps_pool", bufs=2, space="PSUM"))

            wk_sb = persist.tile([P, NHP, KTC, 2, P], FP8)
            wq_sb = persist.tile([P, NHP, KTC, 2, P], FP8)
            wv_sb = persist.tile([P, KTC, 2, OF], FP8)
            xk_sb = persist.tile([P, KTC, 2, S], FP8)
            xq_sb = persist.tile([P, KTC, 2, S], FP8)
            # kh as (hi, lo) fp8 pair, head pairs stacked on partitions;
            # qh plain fp8; v bf16 [tok, of]
            kh = persist.tile([P, NHP, 2, S], FP8)
            qh = persist.tile([P, NHP, S], FP8)
            v_sb = persist.tile([P, NKT, OF], BF16)

            # ---------- producer closures ----------
            def dma_w_hp(w_sb, w_dram, hp):
                def run():
                    nc.sync.dma_start(
                        w_sb[:, hp].rearrange("p kt l n -> p (kt l n)"),
                        w_dram[hp])
                return run

            def dma_wv():
                def run():
                    nc.sync.dma_start(wv_sb[:], wv_r)
                return run

            def dma_x(x_sb, x_r, lo, hi):
                def run():
                    for lv in range(2):
                        nc.sync.dma_start(
                            x_sb[:, :, lv, lo:hi],
                            x_r[:, :, lv, lo:hi])
                return run

            xv_tiles = {}

            def dma_xv(c, lv):
                def run():
                    if lv == 0:
                        xv_tiles[c] = xvpool.tile([P, KTC, 2, 512], FP8,
                                                  tag="xvchunk",
                                                  name=f"xv_{c}")
                    nc.sync.dma_start(
                        xv_tiles[c][:, :, lv, :],
                        xv_r[:, :, lv, c * 512:(c + 1) * 512])
                return run

            # warm-up: the PE runs at 0.65/1.2GHz until ~3us of continuous
            # execution; burn dummy matmuls during the prefix DMAs so the
            # first projections run at full clock
            wu_sb = persist.tile([HD, 2, 640], FP8)

            def warmup(n_mm):
                def run():
                    if n_mm < 0:
                        nc.vector.memset(wu_sb[:], 0)
                        return
                    st = ps_pool.tile([P, 3, 512], F32, tag="st3", bufs=1,
                                      name=f"wu_{n_mm}")
                    for m in range(n_mm):
                        nc.tensor.matmul(
                            st[:, m % 3, :],
                            lhsT=wu_sb[:, :, 0:P],
                            rhs=wu_sb[:, :, P:P + 512],
                            start=True, stop=True,
                            perf_mode=mybir.MatmulPerfMode.DoubleRow,
                            skip_group_check=True,
                        )
                return run

            # split-emission prefix projections: the hi terms only need the
            # lv0 (hi) half of the x chunk, so they start ~1.5us earlier
            _prefix_ps = {}

            def proj_kq_hi(x_sb, w_sb, hp, c, key):
                def run():
                    ps = ps_pool.tile([P, 512], F32, tag="proj", bufs=2,
                                      name=f"pjh_{key}")
                    _prefix_ps[key] = ps
                    n = 0
                    for lw, lx in ((0, 0), (1, 0)):
                        for t in range(NDR):
                            nc.tensor.matmul(
                                ps[:],
                                lhsT=w_sb[:, hp, 2 * t:2 * t + 2, lw, :],
                                rhs=x_sb[:, 2 * t:2 * t + 2, lx,
                                         c * 512:(c + 1) * 512],
                                start=(n == 0), stop=False,
                                perf_mode=mybir.MatmulPerfMode.DoubleRow,
                                skip_group_check=True,
                            )
                            n += 1
                return run

            def proj_kq_lo(x_sb, w_sb, hp, c, dst, split_lo, key):
                def run():
                    ps = _prefix_ps.pop(key)
                    for t in range(NDR):
                        nc.tensor.matmul(
                            ps[:],
                            lhsT=w_sb[:, hp, 2 * t:2 * t + 2, 0, :],
                            rhs=x_sb[:, 2 * t:2 * t + 2, 1,
                                     c * 512:(c + 1) * 512],
                            start=False, stop=(t == NDR - 1),
                            perf_mode=mybir.MatmulPerfMode.DoubleRow,
                            skip_group_check=True,
                        )
                    sl = slice(c * 512, (c + 1) * 512)
                    if split_lo:
                        nc.vector.tensor_copy(out=dst[:, hp, 0, sl], in_=ps[:])
                        nc.vector.tensor_sub(dst[:, hp, 1, sl], ps[:],
                                             dst[:, hp, 0, sl])
                    else:
                        nc.vector.tensor_copy(out=dst[:, hp, sl], in_=ps[:])
                return run

            def dma_x_lv(x_sb, x_r, lo, hi, lv):
                def run():
                    nc.sync.dma_start(
                        x_sb[:, :, lv, lo:hi],
                        x_r[:, :, lv, lo:hi])
                return run

            def proj_kq(x_sb, w_sb, hp, c, dst, split_lo):
                """dst slice [of-pair, tok chunk c] for head-pair hp."""
                def run():
                    ps = ps_pool.tile([P, 512], F32, tag="proj", bufs=2,
                                      name=f"pj_{'k' if split_lo else 'q'}"
                                           f"_{hp}_{c}")
                    n, last = 0, 3 * (NDR + AUGK) - 1
                    for lw, lx in TERMS:
                        for t in range(NDR):
                            nc.tensor.matmul(
                                ps[:],
                                lhsT=w_sb[:, hp, 2 * t:2 * t + 2, lw, :],
                                rhs=x_sb[:, 2 * t:2 * t + 2, lx,
                                         c * 512:(c + 1) * 512],
                                start=(n == 0), stop=(n == last),
                                perf_mode=mybir.MatmulPerfMode.DoubleRow,
                                skip_group_check=True,
                            )
                            n += 1
                        if AUGK:
                            nc.tensor.matmul(
                                ps[:],
                                lhsT=w_sb[:, hp, KTC - 1, lw, :],
                                rhs=x_sb[:, KTC - 1, lx,
                                         c * 512:(c + 1) * 512],
                                start=(n == 0), stop=(n == last),
                                skip_group_check=True,
                            )
                            n += 1
                    sl = slice(c * 512, (c + 1) * 512)
                    if split_lo:
                        nc.vector.tensor_copy(out=dst[:, hp, 0, sl], in_=ps[:])
                        nc.vector.tensor_sub(dst[:, hp, 1, sl], ps[:],
                                             dst[:, hp, 0, sl])
                    else:
                        nc.vector.tensor_copy(out=dst[:, hp, sl], in_=ps[:])
                return run

            _v_ps = {}

            def proj_v(t, part):
                """v_sb[:, t, :] = x-token-tile t @ Wv ([tok, of]).
                part 'hi' takes the two x-hi terms, 'lo' the x-lo term +
                the PSUM drain (finer interleaving in the in-order PE queue,
                and 'hi' only needs the lv0 half of the xv chunk)."""
                def run():
                    xt = xv_tiles[t // 4]
                    tsl = slice((t % 4) * P, (t % 4 + 1) * P)
                    terms = (((0, 0), (1, 0)) if part == "hi" else ((0, 1),))
                    if part == "hi":
                        ps = ps_pool.tile([P, 512], F32, tag="proj", bufs=2,
                                          name=f"pj_v_{t}")
                        _v_ps[t] = ps
                    else:
                        ps = _v_ps.pop(t)
                    n = 0
                    nterm = len(terms) * (NDR + AUGK)
                    for lw, lx in terms:
                        for u in range(NDR):
                            nc.tensor.matmul(
                                ps[:],
                                lhsT=xt[:, 2 * u:2 * u + 2, lx, tsl],
                                rhs=wv_sb[:, 2 * u:2 * u + 2, lw, :],
                                start=(part == "hi" and n == 0),
                                stop=(part == "lo" and n == nterm - 1),
                                perf_mode=mybir.MatmulPerfMode.DoubleRow,
                                skip_group_check=True,
                            )
                            n += 1
                        if AUGK:
                            nc.tensor.matmul(
                                ps[:],
                                lhsT=xt[:, KTC - 1, lx, tsl],
                                rhs=wv_sb[:, KTC - 1, lw, :],
                                start=(part == "hi" and n == 0),
                                stop=(part == "lo" and n == nterm - 1),
                                skip_group_check=True,
                            )
                            n += 1
                    if part == "lo":
                        nc.vector.tensor_copy(out=v_sb[:, t, :], in_=ps[:])
                return run

            # ---------- static schedule ----------
            # jobs: rounds of (h, qcp), k-token axis outermost inside the
            # round: (h, qc = 2*qcp + i, kt); job = h*64 + qcp*32 + kt*2 + i.
            # Round 0 staggers its second q-chunk by 2 k-tiles so the first
            # waves only need qh[qc0] (whose projection finishes first).
            jobs = []
            for kt in range(NKT + 2):
                if kt < NKT:
                    jobs.append((0, 0, kt))
                if kt >= 2:
                    jobs.append((0, 1, kt - 2))
            jobs += [(h, 2 * qcp + i, kt)
                     for h in range(NH) for qcp in range(NQCP)
                     for kt in range(NKT) for i in range(2)
                     if not (h == 0 and qcp == 0)]
            waves = []
            i0 = 0
            size = 3
            while i0 < len(jobs):
                waves.append(jobs[i0:i0 + size])
                i0 += size
                size = 5 - size  # alternate 3, 2

            producers = []  # (due_job, closure, vtile_or_None)
            # prefix + all input DMAs in deadline order (the DMA engines are
            # effectively serial; emission order = transfer order)
            if AUGK == 0:
                producers += [
                    (-99.9, warmup(-1), None),
                    (-99.8, warmup(10), None),
                    (-99.0, dma_w_hp(wk_sb, wk, 0), None),
                    (-98.9, dma_x_lv(xk_sb, xk_r, 0, 512, 0), None),
                    (-98.8, dma_w_hp(wq_sb, wq, 0), None),
                    (-98.7, proj_kq_hi(xk_sb, wk_sb, 0, 0, "k00"), None),
                    (-98.6, dma_x_lv(xk_sb, xk_r, 0, 512, 1), None),
                    (-98.5, proj_kq_lo(xk_sb, wk_sb, 0, 0, kh, True, "k00"),
                     None),
                    (-98.4, dma_x_lv(xq_sb, xq_r, 0, 512, 0), None),
                    (-98.3, warmup(4), None),
                    (-98.2, proj_kq_hi(xq_sb, wq_sb, 0, 0, "q00"), None),
                    (-98.1, dma_x_lv(xq_sb, xq_r, 0, 512, 1), None),
                    (-98.0, proj_kq_lo(xq_sb, wq_sb, 0, 0, qh, False, "q00"),
                     None),
                    (-97.9, dma_x_lv(xq_sb, xq_r, 512, 1024, 0), None),
                    (-97.8, proj_kq_hi(xq_sb, wq_sb, 0, 1, "q01"), None),
                    (-97.7, dma_x_lv(xq_sb, xq_r, 512, 1024, 1), None),
                    (-97.6, proj_kq_lo(xq_sb, wq_sb, 0, 1, qh, False, "q01"),
                     None),
                ]
            else:
                producers += [
                    (-99.0, dma_w_hp(wk_sb, wk, 0), None),
                    (-98.8, dma_x(xk_sb, xk_r, 0, 512), None),
                    (-98.6, dma_w_hp(wq_sb, wq, 0), None),
                    (-98.4, dma_x(xq_sb, xq_r, 0, 512), None),
                    (-98.2, proj_kq(xk_sb, wk_sb, 0, 0, kh, True), None),
                    (-98.0, proj_kq(xq_sb, wq_sb, 0, 0, qh, False), None),
                    (-97.8, dma_x(xq_sb, xq_r, 512, 1024), None),
                    (-97.6, proj_kq(xq_sb, wq_sb, 0, 1, qh, False), None),
                ]
            # earliest-deadline-first input stream; K(0, c1/c2) also use
            # split emission so kh is ready ~1.5us after the lv0 half lands
            if AUGK == 0:
                producers += [
                    (-89.8, dma_x_lv(xk_sb, xk_r, 512, 1024, 0), None),
                    (-89.7, proj_kq_hi(xk_sb, wk_sb, 0, 1, "k01"), None),
                    (-89.6, dma_x_lv(xk_sb, xk_r, 512, 1024, 1), None),
                    (-89.5, proj_kq_lo(xk_sb, wk_sb, 0, 1, kh, True, "k01"),
                     None),
                    (-88.8, dma_x_lv(xk_sb, xk_r, 1024, 1536, 0), None),
                    (-88.7, proj_kq_hi(xk_sb, wk_sb, 0, 2, "k02"), None),
                    (-88.6, dma_x_lv(xk_sb, xk_r, 1024, 1536, 1), None),
                    (-88.5, proj_kq_lo(xk_sb, wk_sb, 0, 2, kh, True, "k02"),
                     None),
                ]
            else:
                producers.append((-89, dma_x(xk_sb, xk_r, 512, 1024), None))
                producers.append((-88, dma_x(xk_sb, xk_r, 1024, 1536), None))
                producers.append((2, proj_kq(xk_sb, wk_sb, 0, 1, kh, True),
                                  None))
                producers.append((10, proj_kq(xk_sb, wk_sb, 0, 2, kh, True),
                                  None))
            producers.append((-87, dma_x(xk_sb, xk_r, 1536, 2048), None))
            producers.append((-86, dma_x(xq_sb, xq_r, 1024, 1536), None))
            producers.append((-85, dma_x(xq_sb, xq_r, 1536, 2048), None))
            producers.append((-84, dma_wv(), None))
            producers.append((-83, dma_xv(0, 0), None))
            producers.append((-82.5, dma_xv(0, 1), None))
            producers.append((-82, dma_xv(1, 0), None))
            producers.append((-81.5, dma_xv(1, 1), None))
            producers.append((10, dma_xv(2, 0), None))
            producers.append((10.5, dma_xv(2, 1), None))
            producers.append((20, dma_xv(3, 0), None))
            producers.append((20.5, dma_xv(3, 1), None))
            # kh chunk c3 needed from kt 12 (job ~25)
            producers.append((18, proj_kq(xk_sb, wk_sb, 0, 3, kh, True),
                              None))
            # Q projections for qc2/3: needed from job 32
            producers.append((24, proj_kq(xq_sb, wq_sb, 0, 2, qh, False),
                              None))
            producers.append((25, proj_kq(xq_sb, wq_sb, 0, 3, qh, False),
                              None))
            # V tiles: paced behind their xv chunk's DMA slot
            for t in range(NKT):
                d = 42 + (t // 4) * 6 + (t % 4)
                producers.append((d, proj_v(t, "hi"), None))
                producers.append((d + 0.5, proj_v(t, "lo"), t))
            for hp in range(1, NHP):
                base = 128 * hp
                producers.append((base - 64, dma_w_hp(wk_sb, wk, hp), None))
                producers.append((base - 62, dma_w_hp(wq_sb, wq, hp), None))
                for c in range(TOKC):
                    producers.append((base + 8 * c - 8,
                                      proj_kq(xk_sb, wk_sb, hp, c, kh, True),
                                      None))
                for qc in range(QC):
                    producers.append((base + 32 * (qc // 2) - 8 + (qc % 2),
                                      proj_kq(xq_sb, wq_sb, hp, qc, qh,
                                              False), None))
            producers.sort(key=lambda e: e[0])
            producers = deque(producers)
            v_emit_wave = {}

            # AV bookkeeping
            av_fifo = deque()  # (job_idx, h, qc, kt, a_t, j_in_wave, wave)
            av_state = {"tile": None, "round": -1}

            def finalize_round(r):
                av = av_state["tile"]
                h, qcp = divmod(r, NQCP)
                o_sb = opool.tile([P, 2, QC, HD], F32, tag="o_sb",
                                  name=f"osb_{r}")
                nc.vector.tensor_scalar_mul(
                    o_sb[:],
                    av[:].rearrange("p (i qt d) -> p i qt d", i=2, qt=QC),
                    1.0 / WS)
                for i in range(2):
                    qc = 2 * qcp + i
                    dst = o[qc * 512:(qc + 1) * 512,
                            h * HD:(h + 1) * HD].rearrange(
                                "(qt p) d -> p qt d", p=P)
                    nc.sync.dma_start(dst, o_sb[:, i])
                av_state["tile"] = None

            def drain_avs(cur_wave, final=False):
                budget = 6  # cap per-wave AV emission so a backlog burst
                # never parks in front of the score stream in the in-order
                # PE queue
                while av_fifo:
                    job, h, qc, kt, a_t, j, w = av_fifo[0]
                    if not final:
                        if budget <= 0:
                            break
                        if w >= cur_wave:
                            break
                        vw = v_emit_wave.get(kt)
                        if vw is None or vw >= cur_wave:
                            break
                        budget -= 1
                    av_fifo.popleft()
                    r = job // RJOBS
                    if r != av_state["round"]:
                        if av_state["tile"] is not None:
                            finalize_round(av_state["round"])
                        av_state["tile"] = ps_pool.tile(
                            [P, 512], F32, tag="av", bufs=1, name=f"av_{r}")
                        av_state["round"] = r
                    av = av_state["tile"]
                    i = qc % 2
                    first = (kt == 0 and i == 0)
                    last = (kt == NKT - 1 and i == 1)
                    for qt in range(4):
                        nc.tensor.matmul(
                            av[:, (i * 4 + qt) * HD:(i * 4 + qt + 1) * HD],
                            lhsT=a_t[:, j, qt * P:(qt + 1) * P],
                            rhs=v_sb[:, kt, h * HD:(h + 1) * HD],
                            start=(first and qt == 0),
                            stop=(last and qt == 3),
                            skip_group_check=True,
                        )

            # ---------- main wave loop ----------
            def drain_producers(w, job_base):
                while producers and producers[0][0] <= job_base + 2:
                    due, closure, vtile = producers.popleft()
                    closure()
                    if vtile is not None:
                        v_emit_wave[vtile] = w

            job_base = 0
            for w, wave in enumerate(waves):
                drain_producers(w, job_base)
                g = len(wave)
                st = ps_pool.tile([P, g, 512], F32, tag=f"st{g}", bufs=1,
                                  name=f"st_{w}")
                for j, (h, qc, kt) in enumerate(wave):
                    hp, pb = h // 2, (h % 2) * HD
                    lhsT = kh[pb:pb + HD, hp, :, kt * P:(kt + 1) * P]
                    for half in range(2):
                        rhs = qh[pb:pb + HD, hp,
                                 qc * 512 + half * 256:
                                 qc * 512 + (half + 1) * 256]
                        rhs = rhs.unsqueeze(1).broadcast_to([HD, 2, 256])
                        nc.tensor.matmul(
                            st[:, j, half * 256:(half + 1) * 256],
                            lhsT=lhsT,
                            rhs=rhs,
                            start=True,
                            stop=True,
                            perf_mode=mybir.MatmulPerfMode.DoubleRow,
                            tile_position=(pb, 0),
                            skip_group_check=True,
                        )
                a_t = apool.tile([P, 3, 512], BF16, tag="a_t", name=f"a_{w}")
                nc.scalar.activation(
                    out=a_t[:, :g, :],
                    in_=st[:],
                    func=mybir.ActivationFunctionType.Sigmoid,
                    scale=0.125 / (WS * WS),
                )
                for j, (h, qc, kt) in enumerate(wave):
                    av_fifo.append((h * 64 + (qc // 2) * 32 + kt * 2
                                    + (qc % 2), h, qc, kt, a_t, j, w))
                drain_avs(w)
                job_base += g
            while producers:
                producers.popleft()[1]()
            drain_avs(0, final=True)
            finalize_round(av_state["round"])

    nc.compile()
    return nc


def _prep_core_inputs(q, k, v, Wq, bq, Wk, bk, Wv, bv, KTC):
    """Host-side shard + transpose + split-fp8 packing. in_maps for 8 cores."""
    import ml_dtypes
    E4 = ml_dtypes.float8_e4m3
    KA = KTC * P
    aug = KA > D

    def split8(a):
        """[R, C] fp32 -> [R, 2, C] fp8 (hi, lo)."""
        hi = a.astype(E4)
        lo = (a - hi.astype(np.float32)).astype(E4)
        return np.ascontiguousarray(np.stack([hi, lo], axis=1))

    def x_t(x_b):  # [S, D] -> [KA, 2, S] fp8
        xt = np.ascontiguousarray(x_b.T)
        if aug:
            pad = np.zeros((KA, S), np.float32)
            pad[:D] = xt
            pad[D] = 1.0
            xt = pad
        return split8(xt)

    def w_kq(W, b, half):  # -> [NHP, P, KTC*2*128] fp8, p-major
        ws = np.ascontiguousarray(W[:, half * OF:(half + 1) * OF]) * WS
        if aug:
            pad = np.zeros((KA, OF), np.float32)
            pad[:D] = ws
            pad[D] = b[half * OF:(half + 1) * OF] * WS
            ws = pad
        s8 = split8(ws)  # [KA, 2, OF]
        pm = s8.reshape(KTC, P, 2, NHP, P).transpose(3, 1, 0, 2, 4)
        return np.ascontiguousarray(pm.reshape(NHP, P, KTC * 2 * P))

    def w_v(W, b, half):  # -> [KA, 2, OF] fp8
        ws = np.ascontiguousarray(W[:, half * OF:(half + 1) * OF]) * WS
        if aug:
            pad = np.zeros((KA, OF), np.float32)
            pad[:D] = ws
            pad[D] = b[half * OF:(half + 1) * OF] * WS
            ws = pad
        return split8(ws)

    xts = {}
    in_maps = []
    for c in range(N_CORES):
        b, half = divmod(c, 2)
        if b not in xts:
            xts[b] = (x_t(q[b]), x_t(k[b]), x_t(v[b]))
        xq_c, xk_c, xv_c = xts[b]
        in_maps.append({
            "xq": xq_c,
            "xk": xk_c,
            "xv": xv_c,
            "wq": w_kq(Wq, bq, half),
            "wk": w_kq(Wk, bk, half),
            "wv": w_v(Wv, bv, half),
        })
    return in_maps


def kernel(q, k, v, Wq, bq, Wk, bk, Wv, bv):
    global last_results
    q = np.ascontiguousarray(np.asarray(q, np.float32))
    k = np.ascontiguousarray(np.asarray(k, np.float32))
    v = np.ascontiguousarray(np.asarray(v, np.float32))
    Wq = np.asarray(Wq, np.float32)
    Wk = np.asarray(Wk, np.float32)
    Wv = np.asarray(Wv, np.float32)
    bq = np.asarray(bq, np.float32)
    bk = np.asarray(bk, np.float32)
    bv = np.asarray(bv, np.float32)

    aug = any(np.any(b_) for b_ in (bq, bk, bv))
    KTC = (D // P) + (1 if aug else 0)

    if KTC not in _cache:
        _cache[KTC] = _build(KTC)
    nc = _cache[KTC]

    in_maps = _prep_core_inputs(q, k, v, Wq, bq, Wk, bk, Wv, bv, KTC)
    res = run_bass_kernel_spmd(nc, in_maps, core_ids=list(range(N_CORES)))
    last_results = res

    out = np.empty((B, S, D), np.float32)
    for c in range(N_CORES):
        b, half = divmod(c, 2)
        out[b, :, half * OF:(half + 1) * OF] = res.results[c]["o"]
    return out
